# revision 6
# baseline (speedup 1.0000x reference)
"""DECO contact-head kernel for 8 trn2 NeuronCores.

Strategy: shard the SMPL-vertex axis (V=6890 -> 8 x 864 padded) across
cores; every core computes all 16 batches for its vertex shard. This
shards the two big weight reads (W_cls 28MB, pos_emb 7MB) and the 59MB
semantic_cont output. The small projector/attention chain is replicated.

Key algebraic facts used:
  - softmax over a length-1 axis == 1.0, so att = ((f@Wc)@Wv)@Wo exactly;
    W_scene/b_scene/Wq/Wk never affect the output.
  - combined @ W_ft1 = att@W1a (per-batch, tiny) + pos_emb@W1b (per-vertex,
    batch-independent -> computed once per core for its shard).
  - mask = (cont > 0.5) == (z > 0) where z = att@W_cls + b_cls, so the mask
    is computed from the pre-sigmoid logits in exact fp32.
"""

import numpy as np

B, DF, D, V, PD, HID, C = 16, 1536, 1024, 6890, 256, 512, 133
NCORES = 8
VS = 864                # vertex shard per core
VPAD = VS * NCORES      # 6912
VT = (512, 352)         # v-tile split inside a shard (both >=256 for f32r)

_cache = {}


def _build_bass():
    from contextlib import ExitStack
    import concourse.bass as bass
    import concourse.tile as tile
    from concourse import bacc, mybir, library_config

    dt = mybir.dt
    F32 = dt.float32
    F32R = dt.float32r
    Alu = mybir.AluOpType
    Act = mybir.ActivationFunctionType

    nc = bacc.Bacc("TRN2", target_bir_lowering=False, debug=False)

    # ---- DRAM I/O (per-core; names keyed by in_maps) ----
    fTh = nc.dram_tensor("fTh", [128, 12, B], F32, kind="ExternalInput")
    Wch = nc.dram_tensor("Wch", [128, 12, D], F32, kind="ExternalInput")
    Wvh = nc.dram_tensor("Wvh", [128, 8, D], F32, kind="ExternalInput")
    Woh = nc.dram_tensor("Woh", [128, 8, D], F32, kind="ExternalInput")
    W1ah = nc.dram_tensor("W1ah", [128, 8, HID], F32R, kind="ExternalInput")
    W1bh = nc.dram_tensor("W1bh", [128, 2, HID], F32R, kind="ExternalInput")
    W2h = nc.dram_tensor("W2h", [128, 4, C], F32R, kind="ExternalInput")
    Wclsh = nc.dram_tensor("Wclsh", [128, 8, VS], F32, kind="ExternalInput")
    posTh = nc.dram_tensor("posTh", [128, 2, VS], F32R, kind="ExternalInput")
    bcont = nc.dram_tensor("bcont", [1, D], F32, kind="ExternalInput")
    bclss = nc.dram_tensor("bclss", [1, VS], F32, kind="ExternalInput")
    bft1c = nc.dram_tensor("bft1c", [128, 4], F32, kind="ExternalInput")
    b2m = nc.dram_tensor("b2m", [128, 1], F32, kind="ExternalInput")
    b2t = nc.dram_tensor("b2t", [5, 1], F32, kind="ExternalInput")

    CONT = nc.dram_tensor("CONT", [B, VS], F32, kind="ExternalOutput")
    SEM = nc.dram_tensor("SEM", [B, C, VS], F32, kind="ExternalOutput")

    def mm(out, lhsT, rhs, start, stop, f32r=False):
        nc.tensor.matmul(out, lhsT, rhs, start=start, stop=stop)

    with tile.TileContext(nc) as tc, ExitStack() as ctx:
        nc.gpsimd.load_library(library_config.proxy)

        const = ctx.enter_context(tc.tile_pool(name="const", bufs=1))
        persist = ctx.enter_context(tc.tile_pool(name="persist", bufs=1))

        ident = const.tile([16, 16], F32)
        from concourse.masks import make_identity
        make_identity(nc, ident[:])
        ones16 = const.tile([1, B], F32)
        nc.any.memset(ones16[:], 1.0)

        # persistent sbuf tensors
        pT_sb = persist.tile([128, 4, VS], F32)        # (pos@W1b)^T, hid-major
        abT_sb = persist.tile([128, 4, B], F32)        # (att@W1a + b_ft1)^T
        attT_sb = persist.tile([128, 8, B], F32)       # att^T chunks
        z_sb = persist.tile([B, VS], F32)              # logits
        maskf_sb = persist.tile([B, VS], F32)          # (z>0) as f32
        mask0_sb = persist.tile([1, B, VS], F32)       # mask rows on partition 0
        dram = ctx.enter_context(tc.tile_pool(name="dram", bufs=1, space="DRAM"))
        mask_dram = dram.tile([B, VS], F32)
        W2_sb = persist.tile([128, 4, C], F32R)
        b2m_sb = persist.tile([128, 1], F32)
        b2t_sb = persist.tile([5, 1], F32)
        bft1_sb = persist.tile([128, 4], F32)

        nc.sync.dma_start(W2_sb[:], W2h.ap())
        nc.sync.dma_start(b2m_sb[:], b2m.ap())
        nc.sync.dma_start(b2t_sb[:], b2t.ap())
        nc.sync.dma_start(bft1_sb[:], bft1c.ap())

        # ============ Phase A: pos-proj pT = (W1b^T @ posT) ============
        with ExitStack() as pctx:
            apool = pctx.enter_context(tc.tile_pool(name="apool", bufs=2))
            apsum = pctx.enter_context(
                tc.tile_pool(name="apsum", bufs=2, space="PSUM"))
            w1b_sb = apool.tile([128, 2, HID], F32R)
            nc.sync.dma_start(w1b_sb[:], W1bh.ap())
            posT_sb = apool.tile([128, 2, VS], F32R)
            nc.sync.dma_start(posT_sb[:], posTh.ap())
            for m in range(4):                       # hid chunk (output part)
                for vo, vn in ((0, 512), (512, 352)):
                    ps = apsum.tile([128, 512], F32, tag="pps")
                    for c in range(2):               # pd chunk (contraction)
                        mm(ps[:, :vn], w1b_sb[:, c, m * 128:(m + 1) * 128],
                           posT_sb[:, c, vo:vo + vn],
                           start=(c == 0), stop=(c == 1), f32r=True)
                    nc.scalar.copy(pT_sb[:, m, vo:vo + vn], ps[:, :vn])

        # ============ Phase B: attention chain (replicated) ============
        with ExitStack() as bctx:
            bpool = bctx.enter_context(tc.tile_pool(name="bpool", bufs=2))
            bmisc = bctx.enter_context(tc.tile_pool(name="bmisc", bufs=2))
            brow = bctx.enter_context(tc.tile_pool(name="brow", bufs=2))
            bpsum = bctx.enter_context(
                tc.tile_pool(name="bpsum", bufs=2, space="PSUM"))
            tpsum = bctx.enter_context(
                tc.tile_pool(name="tpsum", bufs=2, space="PSUM"))

            fT_sb = bmisc.tile([128, 12, B], F32, tag="small")
            nc.sync.dma_start(fT_sb[:], fTh.ap())
            bc_sb = bmisc.tile([1, D], F32, tag="small2")
            nc.sync.dma_start(bc_sb[:], bcont.ap())

            def row_stage(lhsT_chunks, nk, rhs_sb, bias_row):
                """row [B, D] = sum_k lhsT_k.T @ rhs_k (+ ones^T@bias)."""
                row = brow.tile([B, D], F32, tag="row")
                for h in range(2):
                    ps = bpsum.tile([B, 512], F32, tag="bps")
                    for c in range(nk):
                        mm(ps[:], lhsT_chunks(c), rhs_sb[:, c, h * 512:(h + 1) * 512],
                           start=(c == 0), stop=(c == nk - 1 and bias_row is None))
                    if bias_row is not None:
                        mm(ps[:], ones16[:], bias_row[:, h * 512:(h + 1) * 512],
                           start=False, stop=True, f32r=False)
                    nc.vector.tensor_copy(row[:, h * 512:(h + 1) * 512], ps[:])
                return row

            def transpose_row(row, out_sb):
                """row [B, 1024] -> out_sb [128, 8, B] via PE transposes."""
                for c in range(8):
                    tp = tpsum.tile([128, B], F32, tag="tps")
                    nc.tensor.transpose(
                        tp[:], row[:, c * 128:(c + 1) * 128], ident[:])
                    nc.vector.tensor_copy(out_sb[:, c, :], tp[:])

            Wc_sb = bpool.tile([128, 12, D], F32, tag="bigw")
            nc.sync.dma_start(Wc_sb[:], Wch.ap())
            part = row_stage(lambda c: fT_sb[:, c, :], 12, Wc_sb, bc_sb)
            partT = bmisc.tile([128, 8, B], F32, tag="rT")
            transpose_row(part, partT)

            Wv_sb = bpool.tile([128, 8, D], F32, tag="bigw")
            nc.sync.dma_start(Wv_sb[:], Wvh.ap())
            vrow = row_stage(lambda c: partT[:, c, :], 8, Wv_sb, None)
            vT = bmisc.tile([128, 8, B], F32, tag="rT")
            transpose_row(vrow, vT)

            Wo_sb = bpool.tile([128, 8, D], F32, tag="bigw")
            nc.sync.dma_start(Wo_sb[:], Woh.ap())
            att = row_stage(lambda c: vT[:, c, :], 8, Wo_sb, None)
            transpose_row(att, attT_sb)

            # aw1 [B, HID] = att @ W1a  (f32r), then transpose + b_ft1
            W1a_sb = bpool.tile([128, 8, HID], F32R, tag="bigw")
            nc.sync.dma_start(W1a_sb[:], W1ah.ap())
            attT_r = bmisc.tile([128, 8, B], F32R, tag="rTr")
            nc.vector.tensor_copy(attT_r[:], attT_sb[:])
            aw1 = brow.tile([B, HID], F32, tag="row2")
            ps = bpsum.tile([B, HID], F32, tag="bps")
            for c in range(8):
                mm(ps[:], attT_r[:, c, :], W1a_sb[:, c, :],
                   start=(c == 0), stop=(c == 7), f32r=True)
            nc.vector.tensor_copy(aw1[:], ps[:])
            for m in range(4):
                tp = tpsum.tile([128, B], F32, tag="tps")
                nc.tensor.transpose(
                    tp[:], aw1[:, m * 128:(m + 1) * 128], ident[:])
                nc.vector.tensor_scalar(
                    abT_sb[:, m, :], tp[:], bft1_sb[:, m:m + 1], None, Alu.add)

        # ============ Phase C: logits z (exact fp32), mask, cont ============
        with ExitStack() as cctx:
            cpool = cctx.enter_context(tc.tile_pool(name="cpool", bufs=1))
            cpsum = cctx.enter_context(
                tc.tile_pool(name="cpsum", bufs=1, space="PSUM"))
            wcls_sb = cpool.tile([128, 8, VS], F32, tag="wcls")
            nc.sync.dma_start(wcls_sb[:], Wclsh.ap())
            bcls_sb = cpool.tile([1, VS], F32, tag="bcls")
            nc.sync.dma_start(bcls_sb[:], bclss.ap())
            zps = cpsum.tile([B, VS], F32)
            for vo, vn in ((0, 512), (512, 352)):
                for c in range(8):
                    mm(zps[:, vo:vo + vn], attT_sb[:, c, :],
                       wcls_sb[:, c, vo:vo + vn],
                       start=(c == 0), stop=False, f32r=False)
                mm(zps[:, vo:vo + vn], ones16[:], bcls_sb[:, vo:vo + vn],
                   start=False, stop=True, f32r=False)
            nc.vector.tensor_copy(z_sb[:], zps[:])
            nc.vector.tensor_scalar(maskf_sb[:], z_sb[:], 0.0, None, Alu.is_gt)
            # flatten the 16 mask rows onto partition 0 (via DRAM bounce) so
            # the gpsimd partition_broadcast (base-partition-0 only) reads them
            nc.sync.dma_start(mask_dram[:], maskf_sb[:])
            nc.sync.dma_start(mask0_sb[:], mask_dram[:].unsqueeze(0))
            cont_sb = cpool.tile([B, VS], F32, tag="cont")
            zb = const.tile([B, 1], F32)
            nc.any.memset(zb[:], 0.0)
            nc.scalar.activation(cont_sb[:], z_sb[:], Act.Sigmoid, bias=zb[:])
            nc.sync.dma_start(CONT.ap(), cont_sb[:])

        # ============ Phase D: main loop over batches ============
        hpool = ctx.enter_context(tc.tile_pool(name="hpool", bufs=2))
        mpool = ctx.enter_context(tc.tile_pool(name="mpool", bufs=2))
        opool = ctx.enter_context(tc.tile_pool(name="opool", bufs=2))
        otpool = ctx.enter_context(tc.tile_pool(name="otpool", bufs=2))
        pmain = ctx.enter_context(tc.tile_pool(name="pmain", bufs=2, space="PSUM"))
        ptail = ctx.enter_context(tc.tile_pool(name="ptail", bufs=2, space="PSUM"))

        for b in range(B):
            h_t = hpool.tile([128, 4, VS], F32R, tag="h")
            for j in range(4):
                if j < 3:
                    nc.vector.tensor_scalar(
                        h_t[:, j, :], pT_sb[:, j, :], abT_sb[:, j, b:b + 1],
                        0.0, Alu.add, Alu.max)
                else:
                    nc.scalar.activation(
                        h_t[:, j, :], pT_sb[:, j, :], Act.Relu,
                        bias=abT_sb[:, j, b:b + 1])
            mrep = mpool.tile([128, VS], F32, tag="m")
            nc.gpsimd.partition_broadcast(mrep[:], mask0_sb[:, b, :])

            psm = pmain.tile([128, VS], F32, tag="pm")
            pst = ptail.tile([5, VS], F32, tag="pt")
            for vo, vn in ((0, 512), (512, 352)):
                for j in range(4):
                    mm(psm[:, vo:vo + vn], W2_sb[:, j, 0:128],
                       h_t[:, j, vo:vo + vn],
                       start=(j == 0), stop=(j == 3), f32r=True)
                for j in range(4):
                    mm(pst[:, vo:vo + vn], W2_sb[:, j, 128:133],
                       h_t[:, j, vo:vo + vn],
                       start=(j == 0), stop=(j == 3), f32r=True)

            outm = opool.tile([128, VS], F32, tag="om")
            nc.vector.scalar_tensor_tensor(
                outm[:], psm[:], b2m_sb[:], mrep[:], Alu.add, Alu.mult)
            outt0 = otpool.tile([5, VS], F32, tag="ot0")
            nc.scalar.activation(outt0[:], pst[:], Act.Identity, bias=b2t_sb[:])
            outt = otpool.tile([5, VS], F32, tag="ot")
            nc.gpsimd.tensor_tensor(outt[:], outt0[:], mrep[0:5, :], Alu.mult)

            nc.sync.dma_start(SEM.ap()[b, 0:128, :], outm[:])
            nc.sync.dma_start(SEM.ap()[b, 128:133, :], outt[:])

    nc.compile()
    return nc


def _prep_inputs(inputs):
    """Host-side reshape/shard of the full inputs into per-core maps."""
    f32 = np.float32

    def cc(a):
        return np.ascontiguousarray(a, dtype=f32)

    features = inputs["features"]
    W_contact, b_contact = inputs["W_contact"], inputs["b_contact"]
    Wv, Wo = inputs["Wv"], inputs["Wo"]
    W_cls, b_cls = inputs["W_cls"], inputs["b_cls"]
    pos_emb = inputs["pos_emb"]
    W_ft1, b_ft1 = inputs["W_ft1"], inputs["b_ft1"]
    W_ft2, b_ft2 = inputs["W_ft2"], inputs["b_ft2"]

    shared = {
        "fTh": cc(np.asarray(features).T.reshape(12, 128, B).transpose(1, 0, 2)),
        "Wch": cc(np.asarray(W_contact).reshape(12, 128, D).transpose(1, 0, 2)),
        "Wvh": cc(np.asarray(Wv).reshape(8, 128, D).transpose(1, 0, 2)),
        "Woh": cc(np.asarray(Wo).reshape(8, 128, D).transpose(1, 0, 2)),
        "W1ah": cc(np.asarray(W_ft1)[:D].reshape(8, 128, HID).transpose(1, 0, 2)),
        "W1bh": cc(np.asarray(W_ft1)[D:].reshape(2, 128, HID).transpose(1, 0, 2)),
        "W2h": cc(np.asarray(W_ft2).reshape(4, 128, C).transpose(1, 0, 2)),
        "bcont": cc(np.asarray(b_contact).reshape(1, D)),
        "bft1c": cc(np.asarray(b_ft1).reshape(4, 128).T),
        "b2m": cc(np.asarray(b_ft2)[:128].reshape(128, 1)),
        "b2t": cc(np.asarray(b_ft2)[128:].reshape(5, 1)),
    }

    Wcls_p = np.zeros((D, VPAD), f32)
    Wcls_p[:, :V] = np.asarray(W_cls)
    bcls_p = np.zeros((VPAD,), f32)
    bcls_p[:V] = np.asarray(b_cls)
    posT_p = np.zeros((PD, VPAD), f32)
    posT_p[:, :V] = np.asarray(pos_emb).T

    in_maps = []
    for i in range(NCORES):
        s = slice(i * VS, (i + 1) * VS)
        m = dict(shared)
        m["Wclsh"] = cc(Wcls_p[:, s].reshape(8, 128, VS).transpose(1, 0, 2))
        m["bclss"] = cc(bcls_p[s].reshape(1, VS))
        m["posTh"] = cc(posT_p[:, s].reshape(2, 128, VS).transpose(1, 0, 2))
        in_maps.append(m)
    return in_maps


def kernel(**inputs):
    from concourse.bass_utils import run_bass_kernel_spmd

    if "nc" not in _cache:
        _cache["nc"] = _build_bass()
    nc = _cache["nc"]

    in_maps = _prep_inputs(inputs)
    res = run_bass_kernel_spmd(nc, in_maps, list(range(NCORES)))

    cont = np.concatenate(
        [res.results[i]["CONT"] for i in range(NCORES)], axis=1)[:, :V]
    sem = np.concatenate(
        [res.results[i]["SEM"] for i in range(NCORES)], axis=2)[:, :, :V]
    return (np.ascontiguousarray(cont.astype(np.float32)),
            np.ascontiguousarray(sem.astype(np.float32)))


# revision 7
# speedup vs baseline: 1.2408x; 1.2408x over previous
"""DECO contact-head kernel for 8 trn2 NeuronCores.

Strategy: shard the SMPL-vertex axis (V=6890 -> 8 x 864 padded) across
cores; every core computes all 16 batches for its vertex shard. This
shards the two big weight reads (W_cls 28MB, pos_emb 7MB) and the 59MB
semantic_cont output. The small projector/attention chain is replicated.

Key facts used:
  - softmax over a length-1 axis == 1.0, so att = ((f@Wc)@Wv)@Wo exactly;
    W_scene/b_scene/Wq/Wk never affect the output.
  - combined @ W_ft1 = att@W1a (per-batch, tiny) + pos_emb@W1b (per-vertex,
    batch-independent -> computed once per core for its shard).
  - mask = (cont > 0.5) == (z > 0) where z = att@W_cls + b_cls; min |z| on
    this data is ~5e-7, so the whole mask path (chain + logits) runs in
    exact fp32. fp32 matmuls cost 4 cyc/row, so the fp32 stages are
    column-tiled: 4 K-chunks run concurrently in the 4 PE column groups
    (verified ~2.8x), then a 0/1 selector matmul sums the 4 strips.
  - the bulk per-batch matmul (preds = relu(pT+ab) @ W_ft2) runs in
    float32r (e8m11, full PE rate at N>=256) - ~3e-4 rel err, mask-free.
"""

import numpy as np

B, DF, D, V, PD, HID, C = 16, 1536, 1024, 6890, 256, 512, 133
NCORES = 8
VS = 864                # vertex shard per core
VPAD = VS * NCORES      # 6912
VT = ((0, 512), (512, 352))

_cache = {}


def _build_bass():
    from contextlib import ExitStack
    import concourse.bass as bass
    import concourse.tile as tile
    from concourse import bacc, mybir, library_config

    dt = mybir.dt
    F32 = dt.float32
    F32R = dt.float32r
    Alu = mybir.AluOpType
    Act = mybir.ActivationFunctionType

    nc = bacc.Bacc("TRN2", target_bir_lowering=False, debug=False)

    # ---- DRAM I/O (per-core; names keyed by in_maps) ----
    fTh = nc.dram_tensor("fTh", [128, 12, B], F32, kind="ExternalInput")
    Wch = nc.dram_tensor("Wch", [128, 12, D], F32, kind="ExternalInput")
    Wvh = nc.dram_tensor("Wvh", [128, 8, D], F32, kind="ExternalInput")
    Woh = nc.dram_tensor("Woh", [128, 8, D], F32, kind="ExternalInput")
    W1ah = nc.dram_tensor("W1ah", [128, 8, HID], F32R, kind="ExternalInput")
    W1bh = nc.dram_tensor("W1bh", [128, 2, HID], F32R, kind="ExternalInput")
    W2h = nc.dram_tensor("W2h", [128, 4, C], F32R, kind="ExternalInput")
    Wclsh = nc.dram_tensor("Wclsh", [128, 8, VS], F32, kind="ExternalInput")
    posTh = nc.dram_tensor("posTh", [128, 2, VS], F32R, kind="ExternalInput")
    bcont = nc.dram_tensor("bcont", [1, D], F32, kind="ExternalInput")
    bclss = nc.dram_tensor("bclss", [1, VS], F32, kind="ExternalInput")
    bft1c = nc.dram_tensor("bft1c", [128, 4], F32, kind="ExternalInput")
    b2m = nc.dram_tensor("b2m", [128, 1], F32, kind="ExternalInput")
    b2t = nc.dram_tensor("b2t", [5, 1], F32, kind="ExternalInput")
    sel4h = nc.dram_tensor("sel4h", [128, B], F32, kind="ExternalInput")

    CONT = nc.dram_tensor("CONT", [B, VS], F32, kind="ExternalOutput")
    SEM = nc.dram_tensor("SEM", [B, C, VS], F32, kind="ExternalOutput")

    mm = nc.tensor.matmul

    with tile.TileContext(nc) as tc, ExitStack() as ctx:
        nc.gpsimd.load_library(library_config.proxy)

        const = ctx.enter_context(tc.tile_pool(name="const", bufs=1))
        persist = ctx.enter_context(tc.tile_pool(name="persist", bufs=1))
        dram = ctx.enter_context(tc.tile_pool(name="dram", bufs=1, space="DRAM"))

        ident = const.tile([16, 16], F32)
        from concourse.masks import make_identity
        make_identity(nc, ident[:])
        ones16 = const.tile([1, B], F32)
        nc.any.memset(ones16[:], 1.0)
        sel4_sb = const.tile([128, B], F32)
        nc.sync.dma_start(sel4_sb[:], sel4h.ap())

        # persistent sbuf tensors
        pT_sb = persist.tile([128, 4, VS], F32)        # (pos@W1b)^T, hid-major
        abT_sb = persist.tile([128, 4, B], F32)        # (att@W1a + b_ft1)^T
        attT_sb = persist.tile([128, 8, B], F32)       # att^T chunks
        z_sb = persist.tile([B, VS], F32)              # logits
        maskf_sb = persist.tile([B, VS], F32)          # (z>0) as f32
        mask0_sb = persist.tile([1, B, VS], F32)       # mask rows on partition 0
        mask_dram = dram.tile([B, VS], F32)
        W2_sb = persist.tile([128, 4, C], F32R)
        b2m_sb = persist.tile([128, 1], F32)
        b2t_sb = persist.tile([5, 1], F32)
        bft1_sb = persist.tile([128, 4], F32)

        nc.sync.dma_start(W2_sb[:], W2h.ap())
        nc.sync.dma_start(b2m_sb[:], b2m.ap())
        nc.sync.dma_start(b2t_sb[:], b2t.ap())
        nc.sync.dma_start(bft1_sb[:], bft1c.ap())

        def ct_accum(ps, lhsT_chunks, rhs_of, nk, vn):
            """Col-tiled fp32 accumulation: chunk c -> strip c%4 of ps."""
            nc.vector.memset(ps[:], 0.0)
            for r in range((nk + 3) // 4):
                for j in range(4):
                    c = r * 4 + j
                    if c >= nk:
                        continue
                    mm(ps[32 * j:32 * j + B, :vn],
                       lhsT_chunks(c), rhs_of(c),
                       start=(r == 0), stop=(c + 4 >= nk),
                       tile_position=(0, 32 * j), skip_group_check=True)

        # ============ Phase A: pos-proj pT = (W1b^T @ posT) ============
        with ExitStack() as pctx:
            apool = pctx.enter_context(tc.tile_pool(name="apool", bufs=2))
            apsum = pctx.enter_context(
                tc.tile_pool(name="apsum", bufs=2, space="PSUM"))
            w1b_sb = apool.tile([128, 2, HID], F32R)
            nc.sync.dma_start(w1b_sb[:], W1bh.ap())
            posT_sb = apool.tile([128, 2, VS], F32R)
            nc.sync.dma_start(posT_sb[:], posTh.ap())
            for m in range(4):                       # hid chunk (output part)
                for vo, vn in VT:
                    ps = apsum.tile([128, 512], F32, tag="pps")
                    for c in range(2):               # pd chunk (contraction)
                        mm(ps[:, :vn], w1b_sb[:, c, m * 128:(m + 1) * 128],
                           posT_sb[:, c, vo:vo + vn],
                           start=(c == 0), stop=(c == 1))
                    nc.scalar.copy(pT_sb[:, m, vo:vo + vn], ps[:, :vn])

        # ============ Phase B: attention chain (fp32, col-tiled) ============
        with ExitStack() as bctx:
            bpool = bctx.enter_context(tc.tile_pool(name="bpool", bufs=2))
            bmisc = bctx.enter_context(tc.tile_pool(name="bmisc", bufs=2))
            brow = bctx.enter_context(tc.tile_pool(name="brow", bufs=2))
            bpsum = bctx.enter_context(
                tc.tile_pool(name="bpsum", bufs=2, space="PSUM"))
            zpsum = bctx.enter_context(
                tc.tile_pool(name="zpsum", bufs=2, space="PSUM"))
            tpsum = bctx.enter_context(
                tc.tile_pool(name="tpsum", bufs=2, space="PSUM"))

            fT_sb = bmisc.tile([128, 12, B], F32, tag="small")
            nc.sync.dma_start(fT_sb[:], fTh.ap())
            bc_sb = bmisc.tile([1, D], F32, tag="small2")
            nc.sync.dma_start(bc_sb[:], bcont.ap())

            def row_stage(lhsT_chunks, nk, rhs_sb, bias_row):
                """row [B, D] = sum_k lhsT_k.T @ rhs_k (+ ones^T@bias)."""
                row = brow.tile([B, D], F32, tag="row")
                for h in range(2):
                    ps = bpsum.tile([128, 512], F32, tag="strips")
                    ct_accum(ps, lhsT_chunks,
                             lambda c: rhs_sb[:, c, h * 512:(h + 1) * 512],
                             nk, 512)
                    y = bmisc.tile([128, 512], F32, tag="y")
                    nc.vector.tensor_copy(y[:], ps[:])
                    zp = zpsum.tile([B, 512], F32, tag="zp")
                    mm(zp[:], sel4_sb[:], y[:],
                       start=True, stop=(bias_row is None))
                    if bias_row is not None:
                        mm(zp[:], ones16[:], bias_row[:, h * 512:(h + 1) * 512],
                           start=False, stop=True)
                    nc.vector.tensor_copy(row[:, h * 512:(h + 1) * 512], zp[:])
                return row

            def transpose_row(row, out_sb):
                """row [B, 1024] -> out_sb [128, 8, B] via PE transposes."""
                for c in range(8):
                    tp = tpsum.tile([128, B], F32, tag="tps")
                    nc.tensor.transpose(
                        tp[:], row[:, c * 128:(c + 1) * 128], ident[:])
                    nc.vector.tensor_copy(out_sb[:, c, :], tp[:])

            Wc_sb = bpool.tile([128, 12, D], F32, tag="bigw")
            nc.sync.dma_start(Wc_sb[:], Wch.ap())
            part = row_stage(lambda c: fT_sb[:, c, :], 12, Wc_sb, bc_sb)
            partT = bmisc.tile([128, 8, B], F32, tag="rT")
            transpose_row(part, partT)

            Wv_sb = bpool.tile([128, 8, D], F32, tag="bigw")
            nc.sync.dma_start(Wv_sb[:], Wvh.ap())
            vrow = row_stage(lambda c: partT[:, c, :], 8, Wv_sb, None)
            vT = bmisc.tile([128, 8, B], F32, tag="rT")
            transpose_row(vrow, vT)

            Wo_sb = bpool.tile([128, 8, D], F32, tag="bigw")
            nc.sync.dma_start(Wo_sb[:], Woh.ap())
            att = row_stage(lambda c: vT[:, c, :], 8, Wo_sb, None)
            transpose_row(att, attT_sb)

            # aw1 [B, HID] = att @ W1a  (f32r), then transpose + b_ft1
            W1a_sb = bpool.tile([128, 8, HID], F32R, tag="bigw")
            nc.sync.dma_start(W1a_sb[:], W1ah.ap())
            attT_r = bmisc.tile([128, 8, B], F32R, tag="rTr")
            nc.vector.tensor_copy(attT_r[:], attT_sb[:])
            aw1 = brow.tile([B, HID], F32, tag="row2")
            ps = zpsum.tile([B, HID], F32, tag="zp")
            for c in range(8):
                mm(ps[:], attT_r[:, c, :], W1a_sb[:, c, :],
                   start=(c == 0), stop=(c == 7))
            nc.vector.tensor_copy(aw1[:], ps[:])
            for m in range(4):
                tp = tpsum.tile([128, B], F32, tag="tps")
                nc.tensor.transpose(
                    tp[:], aw1[:, m * 128:(m + 1) * 128], ident[:])
                nc.vector.tensor_scalar(
                    abT_sb[:, m, :], tp[:], bft1_sb[:, m:m + 1], None, Alu.add)

        # ======== Phase C: logits z (fp32 col-tiled), mask, cont ========
        with ExitStack() as cctx:
            cpool = cctx.enter_context(tc.tile_pool(name="cpool", bufs=1))
            cmisc = cctx.enter_context(tc.tile_pool(name="cmisc", bufs=2))
            cpsum = cctx.enter_context(
                tc.tile_pool(name="cpsum", bufs=2, space="PSUM"))
            c2psum = cctx.enter_context(
                tc.tile_pool(name="c2psum", bufs=2, space="PSUM"))
            wcls_sb = cpool.tile([128, 8, VS], F32, tag="wcls")
            nc.sync.dma_start(wcls_sb[:], Wclsh.ap())
            bcls_sb = cpool.tile([1, VS], F32, tag="bcls")
            nc.sync.dma_start(bcls_sb[:], bclss.ap())
            for vo, vn in VT:
                ps = cpsum.tile([128, 512], F32, tag="zstrips")
                ct_accum(ps, lambda c: attT_sb[:, c, :],
                         lambda c: wcls_sb[:, c, vo:vo + vn], 8, vn)
                y = cmisc.tile([128, 512], F32, tag="zy")
                nc.vector.tensor_copy(y[:, :vn], ps[:, :vn])
                zp = c2psum.tile([B, 512], F32, tag="zzp")
                mm(zp[:, :vn], sel4_sb[:], y[:, :vn], start=True, stop=False)
                mm(zp[:, :vn], ones16[:], bcls_sb[:, vo:vo + vn],
                   start=False, stop=True)
                nc.vector.tensor_copy(z_sb[:, vo:vo + vn], zp[:, :vn])
            nc.vector.tensor_scalar(maskf_sb[:], z_sb[:], 0.0, None, Alu.is_gt)
            # flatten the 16 mask rows onto partition 0 (via DRAM bounce) so
            # the gpsimd partition_broadcast (base-partition-0 only) reads them
            nc.sync.dma_start(mask_dram[:], maskf_sb[:])
            nc.sync.dma_start(mask0_sb[:], mask_dram[:].unsqueeze(0))
            cont_sb = cpool.tile([B, VS], F32, tag="cont")
            zb = const.tile([B, 1], F32)
            nc.any.memset(zb[:], 0.0)
            nc.scalar.activation(cont_sb[:], z_sb[:], Act.Sigmoid, bias=zb[:])
            nc.sync.dma_start(CONT.ap(), cont_sb[:])

        # ============ Phase D: main loop over batches ============
        hpool = ctx.enter_context(tc.tile_pool(name="hpool", bufs=2))
        mpool = ctx.enter_context(tc.tile_pool(name="mpool", bufs=2))
        opool = ctx.enter_context(tc.tile_pool(name="opool", bufs=2))
        otpool = ctx.enter_context(tc.tile_pool(name="otpool", bufs=2))
        pmain = ctx.enter_context(tc.tile_pool(name="pmain", bufs=2, space="PSUM"))
        ptail = ctx.enter_context(tc.tile_pool(name="ptail", bufs=2, space="PSUM"))

        for b in range(B):
            h_t = hpool.tile([128, 4, VS], F32R, tag="h")
            for j in range(4):
                nc.scalar.activation(
                    h_t[:, j, :], pT_sb[:, j, :], Act.Relu,
                    bias=abT_sb[:, j, b:b + 1])
            mrep = mpool.tile([128, VS], F32, tag="m")
            nc.gpsimd.partition_broadcast(mrep[:], mask0_sb[:, b, :])

            psm = pmain.tile([128, VS], F32, tag="pm")
            pst = ptail.tile([5, VS], F32, tag="pt")
            for vo, vn in VT:
                for j in range(4):
                    mm(psm[:, vo:vo + vn], W2_sb[:, j, 0:128],
                       h_t[:, j, vo:vo + vn],
                       start=(j == 0), stop=(j == 3))
                for j in range(4):
                    mm(pst[:, vo:vo + vn], W2_sb[:, j, 128:133],
                       h_t[:, j, vo:vo + vn],
                       start=(j == 0), stop=(j == 3))

            outm = opool.tile([128, VS], F32, tag="om")
            nc.vector.scalar_tensor_tensor(
                outm[:], psm[:], b2m_sb[:], mrep[:], Alu.add, Alu.mult)
            outt = otpool.tile([5, VS], F32, tag="ot")
            nc.vector.scalar_tensor_tensor(
                outt[:], pst[:], b2t_sb[:], mrep[0:5, :], Alu.add, Alu.mult)

            nc.sync.dma_start(SEM.ap()[b, 0:128, :], outm[:])
            nc.sync.dma_start(SEM.ap()[b, 128:133, :], outt[:])

    nc.compile()
    return nc


def _prep_inputs(inputs):
    """Host-side reshape/shard of the full inputs into per-core maps."""
    f32 = np.float32

    def cc(a):
        return np.ascontiguousarray(a, dtype=f32)

    features = inputs["features"]
    W_contact, b_contact = inputs["W_contact"], inputs["b_contact"]
    Wv, Wo = inputs["Wv"], inputs["Wo"]
    W_cls, b_cls = inputs["W_cls"], inputs["b_cls"]
    pos_emb = inputs["pos_emb"]
    W_ft1, b_ft1 = inputs["W_ft1"], inputs["b_ft1"]
    W_ft2, b_ft2 = inputs["W_ft2"], inputs["b_ft2"]

    sel4 = np.zeros((128, B), f32)
    for j in range(4):
        for b in range(B):
            sel4[32 * j + b, b] = 1.0

    shared = {
        "fTh": cc(np.asarray(features).T.reshape(12, 128, B).transpose(1, 0, 2)),
        "Wch": cc(np.asarray(W_contact).reshape(12, 128, D).transpose(1, 0, 2)),
        "Wvh": cc(np.asarray(Wv).reshape(8, 128, D).transpose(1, 0, 2)),
        "Woh": cc(np.asarray(Wo).reshape(8, 128, D).transpose(1, 0, 2)),
        "W1ah": cc(np.asarray(W_ft1)[:D].reshape(8, 128, HID).transpose(1, 0, 2)),
        "W1bh": cc(np.asarray(W_ft1)[D:].reshape(2, 128, HID).transpose(1, 0, 2)),
        "W2h": cc(np.asarray(W_ft2).reshape(4, 128, C).transpose(1, 0, 2)),
        "bcont": cc(np.asarray(b_contact).reshape(1, D)),
        "bft1c": cc(np.asarray(b_ft1).reshape(4, 128).T),
        "b2m": cc(np.asarray(b_ft2)[:128].reshape(128, 1)),
        "b2t": cc(np.asarray(b_ft2)[128:].reshape(5, 1)),
        "sel4h": sel4,
    }

    Wcls_p = np.zeros((D, VPAD), f32)
    Wcls_p[:, :V] = np.asarray(W_cls)
    bcls_p = np.zeros((VPAD,), f32)
    bcls_p[:V] = np.asarray(b_cls)
    posT_p = np.zeros((PD, VPAD), f32)
    posT_p[:, :V] = np.asarray(pos_emb).T

    in_maps = []
    for i in range(NCORES):
        s = slice(i * VS, (i + 1) * VS)
        m = dict(shared)
        m["Wclsh"] = cc(Wcls_p[:, s].reshape(8, 128, VS).transpose(1, 0, 2))
        m["bclss"] = cc(bcls_p[s].reshape(1, VS))
        m["posTh"] = cc(posT_p[:, s].reshape(2, 128, VS).transpose(1, 0, 2))
        in_maps.append(m)
    return in_maps


def kernel(**inputs):
    from concourse.bass_utils import run_bass_kernel_spmd

    if "nc" not in _cache:
        _cache["nc"] = _build_bass()
    nc = _cache["nc"]

    in_maps = _prep_inputs(inputs)
    res = run_bass_kernel_spmd(nc, in_maps, list(range(NCORES)))

    cont = np.concatenate(
        [res.results[i]["CONT"] for i in range(NCORES)], axis=1)[:, :V]
    sem = np.concatenate(
        [res.results[i]["SEM"] for i in range(NCORES)], axis=2)[:, :, :V]
    return (np.ascontiguousarray(cont.astype(np.float32)),
            np.ascontiguousarray(sem.astype(np.float32)))
